# revision 8
# baseline (speedup 1.0000x reference)
"""Causal multi-head attention (B=2, S=2048, D=1024, H=16) on 8 Trainium2
NeuronCores, tensor-parallel over heads: core c owns heads 2c and 2c+1
(a 128-wide slice of the QKV output dim / Wo input dim).

Dataflow per core (all matmuls float32r = full-rate TF32-like):
  x [4096,1024] --PE-transpose--> xT [1024,4096] (streamed per 512-row s-tile)
  QT/KT/VT [128, 4096] = W*T.T @ xT     (projections, transposed layout)
  per (batch, 512-wide q-tile, 128-wide k-block), heads packed in pairs:
     S^T[k,q]  = KT_blk.T @ QT_tile     (K=64 contraction, 2 heads packed in
                                         PE row-groups via tile_position)
     P^T       = exp(0.125 * (S^T + causal_mask))      (ACT, f32r out)
     ctx^T    += V1_blk.T @ P^T          (V1 = [V | ones] so PSUM row 64
                                          accumulates the softmax denominator)
  ctx normalized via a selector-matmul broadcast of row 64 + DVE reciprocal.
  out_partial = ctx^T.T @ WoT (per 128-row s-block), DMA'd from PSUM.

Host side shards weights across cores, sums the 8 partial outputs (+bias).
"""

import numpy as np

import concourse.mybir as mybir
from concourse import bacc
from concourse.bass_utils import run_bass_kernel_spmd
from concourse.tile import TileContext

B, S, D, H = 2, 2048, 1024, 16
HD = D // H              # 64
BS = B * S               # 4096
NCORES = 8
CP = 128                 # c-dim per core (2 heads x 64)
ST = 512                 # s-tile width for projections
NST = BS // ST           # 8
QTW = 512                # q-tile width for attention
NQT = S // QTW           # 4 per batch
KO = D // 128            # 8 d_in blocks
NKB = S // 128           # 16 k-blocks per batch
NEG = -1.0e30

FP32 = mybir.dt.float32
FP32R = mybir.dt.float32r
AF = mybir.ActivationFunctionType
OP = mybir.AluOpType

_CACHE = {}


def _build():
    nc = bacc.Bacc("TRN2", target_bir_lowering=False, debug=False, num_devices=NCORES)

    x_d = nc.dram_tensor("x_in", [BS, D], FP32R, kind="ExternalInput")
    wq_d = nc.dram_tensor("wq_in", [CP, D], FP32R, kind="ExternalInput")
    wk_d = nc.dram_tensor("wk_in", [CP, D], FP32R, kind="ExternalInput")
    wv_d = nc.dram_tensor("wv_in", [CP, D], FP32R, kind="ExternalInput")
    wo_d = nc.dram_tensor("wo_in", [D, CP], FP32R, kind="ExternalInput")
    id_d = nc.dram_tensor("ident_in", [128, 128], FP32R, kind="ExternalInput")
    tri_d = nc.dram_tensor("tri_in", [128, 128], FP32, kind="ExternalInput")
    sel_d = nc.dram_tensor("sel_in", [65, 128], FP32R, kind="ExternalInput")
    one_d = nc.dram_tensor("ones_in", [128, B * 2 * NKB], FP32R, kind="ExternalInput")
    out_d = nc.dram_tensor("out_p", [BS, D], FP32, kind="ExternalOutput")

    with TileContext(nc) as tc:
        with (
            tc.tile_pool(name="const", bufs=1) as constp,
            tc.tile_pool(name="big", bufs=1) as bigp,
            tc.tile_pool(name="xload", bufs=2) as xloadp,
            tc.tile_pool(name="xt", bufs=2) as xtp,
            tc.tile_pool(name="pt", bufs=4) as ptp,
            tc.tile_pool(name="work", bufs=3) as workp,
            tc.tile_pool(name="ps512", bufs=5, space="PSUM") as ps512,
            tc.tile_pool(name="psacc", bufs=3, space="PSUM") as psacc,
        ):
            # ---- constants -------------------------------------------------
            identt = constp.tile([128, 128], FP32R, tag="ident")
            nc.sync.dma_start(identt[:], id_d.ap())
            trit = constp.tile([128, 128], FP32, tag="tri")
            nc.sync.dma_start(trit[:], tri_d.ap())
            selt = constp.tile([65, 128], FP32R, tag="sel")
            nc.sync.dma_start(selt[:], sel_d.ap())

            # ---- phase 0: weight slices, transposed on PE ------------------
            # wqT/wkT/wvT: [128 (d_in blk), KO, 128 (c)]; woT: [128 (c), KO, 128 (o)]
            wqT = constp.tile([128, KO, 128], FP32R, tag="wqT")
            wkT = constp.tile([128, KO, 128], FP32R, tag="wkT")
            wvT = constp.tile([128, KO, 128], FP32R, tag="wvT")
            woT = constp.tile([128, KO, 128], FP32R, tag="woT")

            for dram, dst, natural in (
                (wq_d, wqT, True),
                (wk_d, wkT, True),
                (wv_d, wvT, True),
                (wo_d, woT, False),
            ):
                wl = xloadp.tile([128, KO, 128], FP32R, tag="wload")
                if natural:
                    # [CP, D] -> [128 (c), KO, 128 (d in blk)]
                    nc.sync.dma_start(
                        wl[:], dram.ap().rearrange("c (ko p) -> c ko p", p=128)
                    )
                else:
                    # [D, CP] -> [128 (o in blk), KO (o blk), 128 (c)]
                    nc.sync.dma_start(
                        wl[:], dram.ap().rearrange("(ko p) c -> p ko c", p=128)
                    )
                for g in range(2):
                    pst = ps512.tile([128, 512], FP32R, tag="ps512")
                    for j in range(4):
                        db = g * 4 + j
                        nc.tensor.transpose(
                            pst[:, j * 128 : (j + 1) * 128], wl[:, db, :], identt[:]
                        )
                    nc.vector.tensor_copy(
                        dst[:, g * 4 : (g + 1) * 4, :].rearrange("p a b -> p (a b)"),
                        pst[:],
                    )

            # ---- phase 1: x transpose + QKV projections --------------------
            qT = bigp.tile([128, NST, ST], FP32R, tag="qT")
            kT = bigp.tile([128, NST, ST], FP32R, tag="kT")
            vT = bigp.tile([128, NST, ST], FP32R, tag="vT")

            for st in range(NST):
                xls = []
                for h in range(2):
                    xl = xloadp.tile([128, 2, D], FP32R, tag="xl")
                    r0 = st * ST + h * 256
                    nc.sync.dma_start(
                        xl[:],
                        x_d.ap()[r0 : r0 + 256, :].rearrange(
                            "(sb p) d -> p sb d", p=128
                        ),
                    )
                    xls.append(xl)
                xt = xtp.tile([128, KO, ST], FP32R, tag="xt")
                for db in range(KO):
                    pst = ps512.tile([128, 512], FP32R, tag="ps512")
                    for sb in range(4):
                        nc.tensor.transpose(
                            pst[:, sb * 128 : (sb + 1) * 128],
                            xls[sb // 2][:, sb % 2, db * 128 : (db + 1) * 128],
                            identt[:],
                        )
                    nc.vector.tensor_copy(xt[:, db, :], pst[:])
                psq = psacc.tile([128, ST], FP32, tag="acc")
                psk = psacc.tile([128, ST], FP32, tag="acc")
                psv = psacc.tile([128, ST], FP32, tag="acc")
                for db in range(KO):
                    first, last = db == 0, db == KO - 1
                    nc.tensor.matmul(
                        psq[:], wqT[:, db, :], xt[:, db, :], start=first, stop=last
                    )
                    nc.tensor.matmul(
                        psk[:], wkT[:, db, :], xt[:, db, :], start=first, stop=last
                    )
                    nc.tensor.matmul(
                        psv[:], wvT[:, db, :], xt[:, db, :], start=first, stop=last
                    )
                nc.vector.tensor_copy(qT[:, st, :], psq[:])
                nc.vector.tensor_copy(kT[:, st, :], psk[:])
                nc.vector.tensor_copy(vT[:, st, :], psv[:])

            # ---- phase 2: V1 = [V | 1] in natural (k-major) layout ---------
            v1 = bigp.tile([128, B, 2, NKB, 65], FP32R, tag="v1")
            nc.sync.dma_start(
                v1[:, :, :, :, 64],
                one_d.ap().rearrange("p (b h k) -> p b h k", b=B, h=2),
            )
            for b in range(B):
                for hp in range(2):
                    for g in range(4):
                        pst = ps512.tile([128, 512], FP32R, tag="ps512")
                        for j in range(4):
                            kb = g * 4 + j
                            sti, off = divmod(b * S + kb * 128, ST)
                            nc.tensor.transpose(
                                pst[:, j * 64 : (j + 1) * 64],
                                vT[64 * hp : 64 * hp + 64, sti, off : off + 128],
                                identt[64 * hp : 64 * hp + 64, 64 * hp : 64 * hp + 64],
                            )
                        nc.vector.tensor_copy(
                            v1[:, b, hp, g * 4 : (g + 1) * 4, 0:64],
                            pst[:, 0:256].rearrange("p (a c) -> p a c", a=4),
                        )

            # ---- phase 3+4: attention + output projection per batch --------
            for b in range(B):
                ctx = bigp.tile([128, S], FP32R, tag=f"ctx{b}")
                for qt in range(NQT):
                    q0 = qt * QTW
                    stq = (b * S + q0) // ST
                    av0 = psacc.tile([65, QTW], FP32, tag="acc", name=f"av0_{b}_{qt}")
                    av1 = psacc.tile([65, QTW], FP32, tag="acc", name=f"av1_{b}_{qt}")
                    av = [av0, av1]
                    nkb = (q0 + QTW) // 128
                    pts = {}
                    for kb in range(nkb + 1):
                        if kb < nkb:
                            k0 = kb * 128
                            j0 = max(0, k0 - q0)
                            stk, offk = divmod(b * S + k0, ST)
                            for hp in range(2):
                                pss = ps512.tile([128, QTW], FP32, tag="ps512")
                                nc.tensor.matmul(
                                    pss[:, j0:QTW],
                                    kT[64 * hp : 64 * hp + 64, stk, offk : offk + 128],
                                    qT[64 * hp : 64 * hp + 64, stq, j0:QTW],
                                    start=True,
                                    stop=True,
                                    tile_position=(64 * hp, 0),
                                )
                                if k0 >= q0:
                                    nc.vector.tensor_tensor(
                                        pss[:, j0 : j0 + 128],
                                        pss[:, j0 : j0 + 128],
                                        trit[:],
                                        OP.add,
                                    )
                                pt = ptp.tile([128, QTW], FP32R, tag="pt")
                                nc.scalar.activation(
                                    pt[:, j0:QTW], pss[:, j0:QTW], AF.Exp, scale=0.125
                                )
                                pts[(kb, hp)] = (pt, j0)
                        if kb > 0:
                            pkb = kb - 1
                            for hp in range(2):
                                pt, j0 = pts.pop((pkb, hp))
                                nc.tensor.matmul(
                                    av[hp][:, j0:QTW],
                                    v1[:, b, hp, pkb, :],
                                    pt[:, j0:QTW],
                                    start=(pkb == 0),
                                    stop=(pkb == nkb - 1),
                                )
                    for hp in range(2):
                        cd = workp.tile([65, QTW], FP32R, tag="cd")
                        nc.scalar.activation(cd[:], av[hp][:], AF.Copy)
                        dbc = ps512.tile([128, QTW], FP32, tag="ps512")
                        nc.tensor.matmul(
                            dbc[:], selt[:], cd[:], start=True, stop=True
                        )
                        rec = workp.tile([64, QTW], FP32, tag="rec")
                        nc.vector.reciprocal(rec[:], dbc[0:64, :])
                        nc.vector.tensor_tensor(
                            ctx[64 * hp : 64 * hp + 64, q0 : q0 + QTW],
                            cd[0:64, :],
                            rec[:],
                            OP.mult,
                        )
                # output projection for this batch
                for sb in range(S // 128):
                    for ot in range(2):
                        po = ps512.tile([128, 512], FP32, tag="ps512")
                        nc.tensor.matmul(
                            po[:],
                            ctx[:, sb * 128 : (sb + 1) * 128],
                            woT[:, ot * 4 : (ot + 1) * 4, :].rearrange(
                                "p a b -> p (a b)"
                            ),
                            start=True,
                            stop=True,
                        )
                        ost = workp.tile([128, 512], FP32, tag="ost")
                        nc.scalar.activation(ost[:], po[:], AF.Copy)
                        r0 = b * S + sb * 128
                        nc.sync.dma_start(
                            out_d.ap()[r0 : r0 + 128, ot * 512 : (ot + 1) * 512],
                            ost[:],
                        )
    nc.compile()
    return nc


def _get_nc():
    if "nc" not in _CACHE:
        _CACHE["nc"] = _build()
    return _CACHE["nc"]


def _consts():
    ident = np.eye(128, dtype=np.float32)
    p = np.arange(128)
    tri = np.where(p[:, None] <= p[None, :], 0.0, NEG).astype(np.float32)
    sel = np.zeros((65, 128), dtype=np.float32)
    sel[64, :] = 1.0
    ones = np.ones((128, B * 2 * NKB), dtype=np.float32)
    return ident, tri, sel, ones


def make_in_maps(inputs):
    x = np.ascontiguousarray(np.asarray(inputs["x"], dtype=np.float32)).reshape(BS, D)
    Wq = np.asarray(inputs["Wq"], dtype=np.float32)
    Wk = np.asarray(inputs["Wk"], dtype=np.float32)
    Wv = np.asarray(inputs["Wv"], dtype=np.float32)
    Wo = np.asarray(inputs["Wo"], dtype=np.float32)

    ident, tri, sel, ones = _consts()
    in_maps = []
    for c in range(NCORES):
        sl = slice(c * CP, (c + 1) * CP)
        in_maps.append(
            {
                "x_in": x,
                "wq_in": np.ascontiguousarray(Wq[sl]),
                "wk_in": np.ascontiguousarray(Wk[sl]),
                "wv_in": np.ascontiguousarray(Wv[sl]),
                "wo_in": np.ascontiguousarray(Wo[:, sl]),
                "ident_in": ident,
                "tri_in": tri,
                "sel_in": sel,
                "ones_in": ones,
            }
        )
    return in_maps


def reduce_outputs(results, bo):
    acc = np.zeros((BS, D), dtype=np.float64)
    for r in results:
        acc += r["out_p"]
    acc += np.asarray(bo, dtype=np.float64)
    return acc.astype(np.float32).reshape(B, S, D)


def kernel(**inputs):
    bo = np.asarray(inputs["bo"], dtype=np.float32)
    in_maps = make_in_maps(inputs)
    nc = _get_nc()
    res = run_bass_kernel_spmd(nc, in_maps, core_ids=list(range(NCORES)))
    return reduce_outputs(res.results, bo)
